# revision 40
# baseline (speedup 1.0000x reference)
"""MoE FFN (8 experts, top-2, GLU) on 8 Trainium2 NeuronCores.

Strategy
--------
Phase 1 (on-device, data-parallel over tokens): each core computes router
logits in fp32 for its 512-token shard and ships, per token, the MASKED
LOGIT DIFFS d_e = 2*l_e - l1 - l2 for the top-2 experts (0 elsewhere).
The softmax-top2-L1 gate c_e = z_e/(z1+z2) equals sigmoid(d_e) exactly,
and the sigmoid is applied in the EXPERT phase where it hides under the
PE -- so the router's top-2 tail is just max8 + 3 DVE ops (no exp, no
reciprocal, no cross-engine hop).  The router stays fp32 so the top-2
SELECTION matches the fp32 reference exactly: reduced-precision logits
flip near-tie tokens and each flip swaps in a whole different expert
output (bf16 x ~6e-2 rel err, fp16 ~2e-2 -- over/at the 2e-2 budget).

Host dispatch (data movement only): tokens are gathered per WEIGHT SLOT.
Each core runs TWO fixed-size expert sub-runs -- slot 0 of S tokens,
slot 1 of CAL-S -- with globally uniform (S, CAL) so one SPMD NEFF
serves all cores, while the host packs which expert owns each of the 16
slots.  Hot experts span two cores' slots, so CAL drops below the
hottest expert's token count (seed-0: loads 988..1063 pack into
CAL=1040, s=536/r=504: two hot experts take 2xS, four mids S+r, two
cold 2xr), saving (1064-1040)/1064 of all FLOPs vs one-expert-per-core.
A small exact-cover DP picks (CAL, S); infeasible inputs fall back to
the single-slot kernel.

Phase 2 (on-device, expert-parallel, bf16 operands / fp32 accumulate):
    h = silu(w1t^T xg) * (v1t^T xg)      [F, CAL]  (h stored bf16)
    yT[hb] = sum_fo w2t[fo,hb]^T h[fo]   [H, CAL]  (output transposed:
             H on partitions, tokens on the free dim -> no token padding
             to 128 in the second GEMM)
    yT *= cb                             (gate bcast along partitions)
bf16 matmuls run at the same 1 cycle/row as fp32r in the cost model but
halve every DMA byte -- and the pipeline head + weight streams are
DMA-latency-bound, not PE-bound.  Error lands at 4.4e-3 vs the 2e-2
gate (fp8 DoubleRow would double PE rate but costs ~5e-2: not viable).

Scheduling notes (what the timeline sim rewards):
 - Per-DMA issue latency (SEQ dispatch + shared HWDGE gen ~0.63us + DGE
   delay ~0.65us) dominates the head: the first-matmul critical path
   ships as ONE host-packed DMA `hd` = [both slots' (fo0,ho0) w1|v1 |
   xg row-block 0], and x stripes follow on sync in consumption order.
 - fo=0 (prologue) and fo=1 both run ho-outer across all 4 slot-chunks,
   with fo<=1 weights split into ho0-3/ho4-7 pieces, so the PE chases
   the serial-DMA-engine stream with sub-0.2us stalls.
 - The tile scheduler hoists dependency-free DMAs to t=0; phase-B w2/cb
   loads are pinned behind phase A via one-element WAW copies so they
   cannot steal the DMA engine during the head.
 - Phase-B psum groups rotate across all three psum pools (8 banks);
   per-slot chunks end in a ~128-wide tail so only a tiny store chain
   (900ns DMA-sem + HWDGE+DGE + drains = the floor) trails the last
   matmul.

Host combine (data movement only): out[slot tokens] += yT slice.

Measured (seed-0 inputs, 8 cores): relative error 4.43e-3; timeline-sim
189.8 us total = router 13.1 us (fp32 xT DMA 6.3 us serial + fixed
latencies) + expert 176.7 us (PE busy 166.6 us = the 1-cycle/row
roofline at CAL=1040, 94.3% occupancy).
"""

import numpy as np
import ml_dtypes

import concourse.bacc as bacc
import concourse.mybir as mybir
import concourse.tile as tile
from concourse.bass_utils import run_bass_kernel_spmd

P = 128
E = 8
H = 1024
F = 2048
T = 4096
NCORES = 8
TSH = T // NCORES  # tokens per core in router phase
HO = H // P  # 8
FO = F // P  # 16
F32 = mybir.dt.float32
BF16 = mybir.dt.bfloat16
BF_NP = ml_dtypes.bfloat16

_NC_CACHE = {}
_W_CACHE = {}
_HEAD_ORDER = ("xg1", "wv0r", "xg2", "xg3", "xg4", "wv0b", "xg5", "xg6",
               "xg7")
_RT_SPLIT = (3,)
WARM_N = 26  # PE p-state warm-up dummy matmuls (128 cycles each)


def _token_chunks(CAL):
    """Split CAL into free-dim chunks <= 512.

    The final chunk is kept small (~128) and the one before it medium, so
    at the very end of phase B the last gate-mul's psum semaphore arrives
    after the DVE has drained the previous mul, and only a tiny store
    trails the final matmul.
    """
    if CAL <= 512:
        return [(0, CAL)]
    chunks = []
    t0 = 0
    while CAL - t0 > 552:
        tl = min(512, CAL - t0 - 552)
        chunks.append((t0, tl))
        t0 += tl
    rem = CAL - t0  # 553..1064 left for the last two chunks
    second = ((rem - 128 + 7) // 8) * 8
    chunks.append((t0, second))
    chunks.append((t0 + second, rem - second))
    return chunks


def _build_router():
    nc = bacc.Bacc("TRN2", target_bir_lowering=False, debug=False,
                   enable_partition_id=False)
    xT = nc.dram_tensor("xT", [H, TSH], F32, kind="ExternalInput")
    rwT = nc.dram_tensor("rwT", [H, E], F32, kind="ExternalInput")
    c_out = nc.dram_tensor("c", [TSH, E], F32, kind="ExternalOutput")
    NT = TSH // P  # token blocks
    with tile.TileContext(nc) as tc:
        with tc.tile_pool(name="xp", bufs=1) as xp, \
             tc.tile_pool(name="wp", bufs=1) as wp, \
             tc.tile_pool(name="sp", bufs=4) as sp, \
             tc.tile_pool(name="cp", bufs=1) as cp, \
             tc.tile_pool(name="ps", bufs=4, space="PSUM") as ps:
            rw = wp.tile([P, HO, E], F32)
            # rw rides the otherwise-idle Act queue so xt0 leads on sync.
            nc.scalar.dma_start(rw[:], xT_rw_src(xT, rwT))
            # x tiles: blocks listed in _RT_SPLIT ride in two half
            # tiles so their first matmuls overlap the second half's
            # transfer (the top-2 tail chain hangs off the LAST block).
            xts = []
            for tt in range(NT):
                if tt not in _RT_SPLIT:
                    xt = xp.tile([P, HO, P], F32, tag=f"xt{tt}",
                                 name=f"xt{tt}")
                    nc.sync.dma_start(
                        xt[:],
                        xT.ap()[:, tt * P:(tt + 1) * P].rearrange(
                            "(ho p) t -> p ho t", p=P))
                    xts.append((xt,))
                else:
                    HH = HO // 2
                    xta = xp.tile([P, HH, P], F32, tag=f"xt{tt}a",
                                  name=f"xt{tt}a")
                    nc.sync.dma_start(
                        xta[:],
                        xT.ap()[0:HH * P, tt * P:(tt + 1) * P].rearrange(
                            "(ho p) t -> p ho t", p=P))
                    xtb = xp.tile([P, HH, P], F32, tag=f"xt{tt}b",
                                  name=f"xt{tt}b")
                    nc.sync.dma_start(
                        xtb[:],
                        xT.ap()[HH * P:H, tt * P:(tt + 1) * P].rearrange(
                            "(ho p) t -> p ho t", p=P))
                    xts.append((xta, xtb))
            pls = [ps.tile([P, E], F32, tag="pl", name=f"pl{tt}")
                   for tt in range(NT)]
            for tt in range(NT):
                for ho in range(HO):
                    parts = xts[tt]
                    HH = HO // len(parts)
                    nc.tensor.matmul(pls[tt][:],
                                     parts[ho // HH][:, ho % HH, :],
                                     rw[:, ho, :],
                                     start=(ho == 0), stop=(ho == HO - 1))
            # Ship d_e = (2*l_e - l1 - l2) masked to the top-2: the gate
            # c_e = z_e/(z1+z2) equals sigmoid(d_e), applied later in the
            # expert phase where it is hidden under the PE.  This keeps
            # the whole top-2 tail on the DVE (no exp / reciprocal / Act
            # hop) -- 4 ops off the last token block's critical chain.
            cgall = cp.tile([P, NT, E], F32)
            for tt in range(NT):
                lt = sp.tile([P, E], F32, tag="lt")
                nc.vector.tensor_copy(lt[:], pls[tt][:])
                m8 = sp.tile([P, 8], F32, tag="m8")
                nc.vector.max(m8[:], lt[:])
                s2 = sp.tile([P, 1], F32, tag="s2")
                nc.vector.tensor_add(s2[:], m8[:, 0:1], m8[:, 1:2])
                d = sp.tile([P, E], F32, tag="d")
                nc.vector.scalar_tensor_tensor(
                    d[:], lt[:], s2[:, 0:1], lt[:],
                    op0=mybir.AluOpType.subtract, op1=mybir.AluOpType.add)
                nc.vector.scalar_tensor_tensor(
                    cgall[:, tt, :], lt[:], m8[:, 1:2], d[:],
                    op0=mybir.AluOpType.is_ge, op1=mybir.AluOpType.mult)
                nc.sync.dma_start(
                    c_out.ap()[tt * P:(tt + 1) * P, :], cgall[:, tt, :])
    nc.compile()
    return nc


def xT_rw_src(xT, rwT):
    return rwT.ap().rearrange("(ho p) e -> p ho e", p=P)


def _build_expert(C, CA):
    CAL = min(C, ((CA + 7) // 8) * 8)
    chunks = _token_chunks(CAL)
    main_path = len(chunks) <= 3
    c0w = chunks[0][1]  # width of the head x stripe packed into hd
    nc = bacc.Bacc("TRN2", target_bir_lowering=False, debug=False,
                   enable_partition_id=False)
    # hd packs the whole first-matmul critical path into ONE DMA:
    # [w1(fo0,ho0) | v1(fo0,ho0) | xg rows 0:128 cols 0:c0w].
    hd = nc.dram_tensor("hd", [P, 2 * P + c0w], BF16, kind="ExternalInput")
    xgT = nc.dram_tensor("xgT", [H, C], BF16, kind="ExternalInput")
    cb = nc.dram_tensor("cb", [P, CAL], F32, kind="ExternalInput")
    wvt = nc.dram_tensor("wvt", [FO, P, 2, HO, P], BF16, kind="ExternalInput")
    w2t = nc.dram_tensor("w2t", [FO, P, HO, P], BF16, kind="ExternalInput")
    y = nc.dram_tensor("y", [H, C], BF16, kind="ExternalOutput")
    with tile.TileContext(nc) as tc:
        with tc.tile_pool(name="xp", bufs=1) as xp, \
             tc.tile_pool(name="hp", bufs=1) as hp, \
             tc.tile_pool(name="wp", bufs=4) as wp, \
             tc.tile_pool(name="w2p", bufs=16) as w2p, \
             tc.tile_pool(name="cp", bufs=1) as cp, \
             tc.tile_pool(name="scp", bufs=2) as scp, \
             tc.tile_pool(name="yp", bufs=2) as yp, \
             tc.tile_pool(name="ps", bufs=3, space="PSUM") as ps, \
             tc.tile_pool(name="psb", bufs=2, space="PSUM") as psb:

            # --- DMA schedule.  Tile dep granularity is per-tile; the DMA
            # engine serves transfers in ready order and the shared HWDGE
            # generator (one per ~630ns) serializes the sync+Act queues,
            # so the head is choreographed: hd first on sync, late xg
            # stripes on Act, early-arriving xg5-7 + phase-B weights on
            # the gpsimd SWDGE path. ---
            hdt = xp.tile([P, 2 * P + c0w], BF16, tag="hd", name="hdt")
            nc.sync.dma_start(hdt[:], hd.ap())

            # xg rows 128..1023 ride in three multi-h-block DMAs (per-DMA
            # dispatch+descriptor-gen latency, not bandwidth, is what
            # limits the pipeline head).
            def load_xg_group(lo, n, q):
                xg = xp.tile([P, n, CAL], BF16, tag=f"xgg{lo}",
                             name=f"xgg{lo}")
                q(xg[:], xgT.ap()[lo * P:(lo + n) * P, :CAL].rearrange(
                    "(j p) t -> p j t", p=P))
                return xg

            xg0b = None
            if CAL > c0w:
                xg0b = xp.tile([P, CAL - c0w], BF16, tag="xg0b", name="xg0b")
                nc.scalar.dma_start(xg0b[:], xgT.ap()[0:P, c0w:CAL])
            # Head-stream issue order on sync (empirically tuned against
            # the timeline sim; see _HEAD_ORDER).
            xg_tiles = [None] * HO
            wv0r = wv0b = None
            for item in _HEAD_ORDER:
                if item == "wv0r":
                    wv0r = wp.tile([P, 2, 3, P], BF16, tag="wv0r",
                                   name="wv0r")
                    nc.sync.dma_start(wv0r[:], wvt.ap()[0, :, :, 1:4, :])
                elif item == "wv0b":
                    wv0b = wp.tile([P, 2, 4, P], BF16, tag="wv0b",
                                   name="wv0b")
                    nc.sync.dma_start(wv0b[:], wvt.ap()[0, :, :, 4:8, :])
                else:
                    ho = int(item[2:])
                    xg_tiles[ho] = load_xg_group(ho, 1, nc.sync.dma_start)

            def wv0_slice(m, ho):
                if ho == 0:
                    return hdt[:, m * P:(m + 1) * P]
                if ho < 4:
                    return wv0r[:, m, ho - 1, :]
                return wv0b[:, m, ho - 4, :]

            def load_wv(fo):
                wv = wp.tile([P, 2, HO, P], BF16, tag="wv", name=f"wv{fo}")
                nc.sync.dma_start(wv[:], wvt.ap()[fo])
                return wv

            wvs = {fo: load_wv(fo) for fo in (1, 2)}

            def xg_slice(ho, t0, tl):
                if ho == 0:
                    if t0 < c0w:
                        return hdt[:, 2 * P + t0:2 * P + t0 + tl]
                    return xg0b[:, t0 - c0w:t0 - c0w + tl]
                return xg_tiles[ho][:, 0, t0:t0 + tl]
            h = hp.tile([P, FO, CAL], BF16)

            # Phase-B inputs load on the gpsimd queue, gated behind an
            # artificial dependency on fo=2's h (issued mid-loop, below)
            # so their transfers cannot outrun the phase-A critical stream
            # on the shared DMA engine (transfers are served in ready
            # order, and these are not needed until ~115us in).
            cbt = cp.tile([P, CAL], F32)
            w2s = [w2p.tile([P, HO, P], BF16, tag="w2", name=f"w2_{fo}")
                   for fo in range(FO)]

            def issue_phaseb_loads():
                # The tile scheduler hoists dependency-free DMAs to t=0,
                # where they would contend with the phase-A critical loads
                # on the shared DMA engine.  A one-element copy from fo=1's
                # h INTO each target tile gives every load a real WAW
                # dependency, pinning the transfers to ~25us+ (the engine
                # is idle there; phase B needs them only at ~115us).
                nc.gpsimd.tensor_copy(cbt[:, 0:1], h[:, 1, 0:1])
                nc.gpsimd.dma_start(cbt[:], cb.ap())
                for fo in range(FO):
                    nc.gpsimd.tensor_copy(w2s[fo][:, 0, 0:1], h[:, 1, 0:1])
                    nc.gpsimd.dma_start(w2s[fo][:], w2t.ap()[fo])

            def glu_tail(fo, t0, tl, p1, p2):
                sc = scp.tile([P, 512], F32, tag="sc", name="sc")[:, :tl]
                nc.scalar.activation(sc, p1,
                                     mybir.ActivationFunctionType.Silu)
                nc.vector.tensor_mul(h[:, fo, t0:t0 + tl], sc, p2)

            if main_path:
                # --- Phase A prologue: fo=0 ho-outer so the PE chases the
                # streaming xg stripes (bf16 stripes outpace the PE, so no
                # extra interleaved work is needed). ---
                ps1s = [ps.tile([P, 512], F32, tag="ps1",
                                name=f"ps1_{i}")[:, :tl]
                        for i, (t0, tl) in enumerate(chunks)]
                ps2s = [ps.tile([P, 512], F32, tag="ps2",
                                name=f"ps2_{i}")[:, :tl]
                        for i, (t0, tl) in enumerate(chunks)]
                for ho in range(HO):
                    st, sp_ = (ho == 0), (ho == HO - 1)
                    for i, (t0, tl) in enumerate(chunks):
                        nc.tensor.matmul(ps1s[i], wv0_slice(0, ho),
                                         xg_slice(ho, t0, tl),
                                         start=st, stop=sp_)
                        nc.tensor.matmul(ps2s[i], wv0_slice(1, ho),
                                         xg_slice(ho, t0, tl),
                                         start=st, stop=sp_)
                for i, (t0, tl) in enumerate(chunks):
                    glu_tail(0, t0, tl, ps1s[i], ps2s[i])

                # --- Phase A steady state ---
                for fo in range(1, FO):
                    wv = wvs[fo] if fo in wvs else load_wv(fo)
                    for i, (t0, tl) in enumerate(chunks):
                        # fo=1 runs on the psb pool (idle until phase B) so
                        # it does not wait on fo=0's psum rotation.
                        pp = psb if fo == 1 else ps
                        t1, t2 = ("psy", "psy") if fo == 1 else ("ps1", "ps2")
                        p1 = pp.tile([P, 512], F32, tag=t1,
                                     name="p1")[:, :tl]
                        p2 = pp.tile([P, 512], F32, tag=t2,
                                     name="p2")[:, :tl]
                        for ho in range(HO):
                            st, sp_ = (ho == 0), (ho == HO - 1)
                            nc.tensor.matmul(p1, wv[:, 0, ho, :],
                                             xg_slice(ho, t0, tl),
                                             start=st, stop=sp_)
                            nc.tensor.matmul(p2, wv[:, 1, ho, :],
                                             xg_slice(ho, t0, tl),
                                             start=st, stop=sp_)
                        glu_tail(fo, t0, tl, p1, p2)
                    if fo == 2:
                        issue_phaseb_loads()
            else:
                # psum-budget fallback: chunk-serial accumulation
                for fo in range(FO):
                    wv = (None if fo == 0
                          else wvs[fo] if fo in wvs else load_wv(fo))
                    for i, (t0, tl) in enumerate(chunks):
                        p1 = ps.tile([P, 512], F32, tag="ps1",
                                     name="p1")[:, :tl]
                        p2 = ps.tile([P, 512], F32, tag="ps2",
                                     name="p2")[:, :tl]
                        for ho in range(HO):
                            st, sp_ = (ho == 0), (ho == HO - 1)
                            l1 = wv0_slice(0, ho) if fo == 0 else wv[:, 0, ho, :]
                            l2 = wv0_slice(1, ho) if fo == 0 else wv[:, 1, ho, :]
                            nc.tensor.matmul(p1, l1, xg_slice(ho, t0, tl),
                                             start=st, stop=sp_)
                            nc.tensor.matmul(p2, l2, xg_slice(ho, t0, tl),
                                             start=st, stop=sp_)
                        glu_tail(fo, t0, tl, p1, p2)
                    if fo == 2:
                        issue_phaseb_loads()

            # gates arrive as logit diffs; sigmoid on the Act engine,
            # pinned after the last silu via a WAW copy on cbs
            cbs = cp.tile([P, CAL], F32, tag="cbs", name="cbs")
            nc.scalar.copy(cbs[:, 0:1], h[:, FO - 1, 0:1])
            nc.scalar.activation(cbs[:], cbt[:],
                                 mybir.ActivationFunctionType.Sigmoid)

            # --- Phase B: yT[hb] = (sum_fo w2[fo,hb]^T h[fo]) * cb ---
            # psy rotates across all three psum pools so the gate-mul
            # never stalls the next accumulation group (a stall would also
            # reset the PE p-state ramp); the last hb stores per-chunk
            # (spread over queues) so only a tiny store trails the final
            # matmul.
            pool_cycle = [(psb, "psy"), (ps, "ps1"), (ps, "ps2")]
            gi = 0
            for hb in range(HO):
                yt = yp.tile([P, CAL], BF16, tag="yt", name=f"yt{hb}")
                last_hb = (hb == HO - 1)
                for ci, (t0, tl) in enumerate(chunks):
                    pool, ptag = pool_cycle[gi % 3]
                    gi += 1
                    psy = pool.tile([P, 512], F32, tag=ptag,
                                    name="psy")[:, :tl]
                    for fo in range(FO):
                        nc.tensor.matmul(psy, w2s[fo][:, hb, :],
                                         h[:, fo, t0:t0 + tl],
                                         start=(fo == 0), stop=(fo == FO - 1))
                    nc.vector.tensor_mul(yt[:, t0:t0 + tl], psy,
                                         cbs[:, t0:t0 + tl])
                    if last_hb:
                        nc.sync.dma_start(
                            y.ap()[hb * P:(hb + 1) * P, t0:t0 + tl],
                            yt[:, t0:t0 + tl])
                if not last_hb:
                    nc.sync.dma_start(y.ap()[hb * P:(hb + 1) * P, 0:CAL],
                                      yt[:])
    nc.compile()
    return nc


def _slot_chunks(lo, width):
    """Chunks for one slot: <=512 each, last ~144 when possible (the
    144 tail empirically minimizes the end-of-phase-B store chain)."""
    TAIL = 144
    if width <= 512:
        if width > 256:
            first = ((width - TAIL + 7) // 8) * 8
            return [(lo, first), (lo + first, width - first)]
        return [(lo, width)]
    out = []
    t0 = lo
    rem = width
    while rem > 512 + TAIL // 2 + 40:
        tl = min(512, rem - (512 + TAIL // 2 + 40))
        out.append((t0, tl))
        t0 += tl
        rem -= tl
    second = ((rem - TAIL + 7) // 8) * 8
    out.append((t0, second))
    out.append((t0 + second, rem - second))
    return out


def _build_expert2(C, CAL, S):
    """Two-slot expert kernel: tokens [0,S) use weight slot 0, [S,CAL)
    slot 1.  Globally uniform (SPMD) shapes; the host packs which expert
    each slot holds, so hot experts span two cores' slots and CAL drops
    below the hottest expert's token count.

    Head structure (pass-AB): fo=0 alone consumes ~450 B/ns of weights+
    activations while the DMA engines deliver 360 B/ns, so a plain
    fo-serial head stalls the PE.  Instead the head runs two merged
    ho-outer passes -- pass A = slot0 x {fo0, fo1}, pass B = slot1 x
    {fo0, fo1} -- each needing exactly the 8 psum banks and consuming
    ~296 B/ns, which the DMA stream can sustain.  xg stripes are loaded
    per-slot so each pass only pulls the columns it reads."""
    ch0 = _slot_chunks(0, S)
    ch1 = _slot_chunks(S, CAL - S)
    RW = CAL - S  # slot1 width
    nc = bacc.Bacc("TRN2", target_bir_lowering=False, debug=False,
                   enable_partition_id=False)
    hd = nc.dram_tensor("hd", [P, 4 * P + S], BF16, kind="ExternalInput")
    hd2 = nc.dram_tensor("hd2", [P, 4 * P + 2 * S], BF16,
                         kind="ExternalInput")
    wvh0 = nc.dram_tensor("wvh0", [3, P, 8 * P], BF16,
                          kind="ExternalInput")
    wvh1 = nc.dram_tensor("wvh1", [4, P, 8 * P], BF16,
                          kind="ExternalInput")
    xgT = nc.dram_tensor("xgT", [H, C], BF16, kind="ExternalInput")
    cb = nc.dram_tensor("cb", [P, CAL], F32, kind="ExternalInput")
    wvt = nc.dram_tensor("wvt", [2, FO, P, 2, HO, P], BF16,
                         kind="ExternalInput")
    w2t = nc.dram_tensor("w2t", [2, FO, P, HO, P], BF16,
                         kind="ExternalInput")
    y = nc.dram_tensor("y", [H, C], BF16, kind="ExternalOutput")
    with tile.TileContext(nc) as tc:
        with tc.tile_pool(name="xp", bufs=1) as xp, \
             tc.tile_pool(name="hp", bufs=1) as hp, \
             tc.tile_pool(name="wp", bufs=6) as wp, \
             tc.tile_pool(name="wh", bufs=1) as wh, \
             tc.tile_pool(name="w2p", bufs=32) as w2p, \
             tc.tile_pool(name="cp", bufs=1) as cp, \
             tc.tile_pool(name="scp", bufs=4) as scp, \
             tc.tile_pool(name="yp", bufs=3) as yp, \
             tc.tile_pool(name="zp", bufs=1) as zp, \
             tc.tile_pool(name="pp", bufs=8, space="PSUM") as pp:

            # --- PE p-state warm-up.  The cost model runs the PE at half
            # clock until 3us of continuous busy time.  Dependency-free
            # dummy matmuls keep the PE busy from ~1.1us so the real
            # stream (first dep ready ~3.4us) starts at full clock.  The
            # warm psum tile shares the pp rotation; its real successor's
            # first matmul has start=True (accumulator reset), so the
            # garbage never escapes.
            zl = zp.tile([P, P], BF16, tag="zl", name="zl")
            nc.gpsimd.memset(zl[:], 0.0)
            warm_ps = pp.tile([P, 512], F32, tag="pp", name="warm")
            for _ in range(WARM_N):
                nc.tensor.matmul(warm_ps[:, 0:P], zl[:], zl[:],
                                 start=True, stop=True)

            # --- Head DMA stream, in PE need-order on the sync queue.
            # Every head DMA is sized >= ~650ns of transfer (the shared
            # HWDGE generator issues one DMA per ~632ns, so smaller DMAs
            # make delivery generator-limited, not bandwidth-limited).
            # hd/hd2 are host-packed so ho0-2 of slot0 (weights fo0,fo1 +
            # xg rows 0-2) arrive in two large DMAs. ---
            hdt = xp.tile([P, 4 * P + S], BF16, tag="hd", name="hdt")
            nc.sync.dma_start(hdt[:], hd.ap())
            hd2t = xp.tile([P, 4 * P + 2 * S], BF16, tag="hd2",
                           name="hd2t")
            nc.sync.dma_start(hd2t[:], hd2.ap())

            def load_wv_pair(src, j, name):
                # host-packed fo{0,1} x {w1,v1} x 2-ho bundle: [P, 1024]
                t = wh.tile([P, 8 * P], BF16, tag=name, name=name)
                nc.sync.dma_start(t[:], src.ap()[j])
                return t

            def load_xg_pair(klo, n, lo, w, name):
                # xg row-blocks klo..klo+n-1, cols [lo, lo+w)
                xg = xp.tile([P, n, w], BF16, tag=name, name=name)
                nc.sync.dma_start(
                    xg[:], xgT.ap()[klo * P:(klo + n) * P, lo:lo + w]
                    .rearrange("(j p) t -> p j t", p=P))
                return xg

            wva_23 = load_wv_pair(wvh0, 0, "wva23")
            xga_34 = load_xg_pair(3, 2, 0, S, "xga34")
            wva_45 = load_wv_pair(wvh0, 1, "wva45")
            xga_56 = load_xg_pair(5, 2, 0, S, "xga56")
            wva_67 = load_wv_pair(wvh0, 2, "wva67")
            xga_7 = load_xg_pair(7, 1, 0, S, "xga7")
            # slot1 (pass B) stream: lands while pass A computes
            xgb_p, wvb_p = [], []
            for j in range(4):
                xgb_p.append(load_xg_pair(2 * j, 2, S, RW, f"xgb{j}"))
                wvb_p.append(load_wv_pair(wvh1, j, f"wvb{j}"))

            def load_wv(sl, fo):
                wv = wp.tile([P, 2, HO, P], BF16, tag="wv",
                             name=f"wv{sl}_{fo}")
                nc.sync.dma_start(wv[:], wvt.ap()[sl, fo])
                return wv

            def wv01_slice(sl, fo, m, ho):
                if sl == 0:
                    if ho == 0:
                        return hdt[:, (2 * fo + m) * P:(2 * fo + m + 1) * P]
                    if ho == 1:
                        return hd2t[:, (2 * fo + m) * P:
                                    (2 * fo + m + 1) * P]
                    t = (wva_23, wva_45, wva_67)[(ho - 2) // 2]
                else:
                    t = wvb_p[ho // 2]
                j = ((fo * 2 + m) * 2 + ho % 2) * P
                return t[:, j:j + P]

            def xg_slice(ho, t0, tl):
                if t0 < S:
                    if ho == 0:
                        return hdt[:, 4 * P + t0:4 * P + t0 + tl]
                    if ho == 1:
                        return hd2t[:, 4 * P + t0:4 * P + t0 + tl]
                    if ho == 2:
                        return hd2t[:, 4 * P + S + t0:4 * P + S + t0 + tl]
                    if ho == 7:
                        return xga_7[:, 0, t0:t0 + tl]
                    t = (xga_34, xga_56)[(ho - 3) // 2]
                    return t[:, (ho - 3) % 2, t0:t0 + tl]
                u = t0 - S
                return xgb_p[ho // 2][:, ho % 2, u:u + tl]
            h = hp.tile([P, FO, CAL], BF16)

            cbt = cp.tile([P, CAL], F32)
            w2s = [[w2p.tile([P, HO, P], BF16, tag="w2",
                             name=f"w2_{sl}_{fo}") for fo in range(FO)]
                   for sl in range(2)]

            def issue_phaseb_loads():
                nc.gpsimd.tensor_copy(cbt[:, 0:1], h[:, 3, 0:1])
                nc.gpsimd.dma_start(cbt[:], cb.ap())
                for sl in range(2):
                    for fo in range(FO):
                        nc.gpsimd.tensor_copy(w2s[sl][fo][:, 0, 0:1],
                                              h[:, 3, 0:1])
                        nc.gpsimd.dma_start(w2s[sl][fo][:],
                                            w2t.ap()[sl, fo])

            cbs = cp.tile([P, CAL], F32, tag="cbs", name="cbs")

            def glu_tail(fo, t0, tl, p1, p2):
                sc = scp.tile([P, 512], F32, tag="sc", name="sc")[:, :tl]
                nc.scalar.activation(sc, p1,
                                     mybir.ActivationFunctionType.Silu)
                nc.vector.tensor_mul(h[:, fo, t0:t0 + tl], sc, p2)

            # --- Pass A (slot0 x fo{0,1}) and pass B (slot1), ho-outer.
            # Each pass holds 2 fo x 2 chunks x 2 = all 8 psum banks.
            def merged_pass(sl, chs, lo):
                grp = []
                for fo in (0, 1):
                    for (t0, tl) in chs:
                        p1 = pp.tile([P, 512], F32, tag="pp",
                                     name=f"pa{sl}_{fo}")[:, :tl]
                        p2 = pp.tile([P, 512], F32, tag="pp",
                                     name=f"pb{sl}_{fo}")[:, :tl]
                        grp.append((fo, t0, tl, p1, p2))
                for ho in range(HO):
                    st, sp_ = (ho == 0), (ho == HO - 1)
                    for (fo, t0, tl, p1, p2) in grp:
                        nc.tensor.matmul(p1, wv01_slice(sl, fo, 0, ho),
                                         xg_slice(ho, t0, tl),
                                         start=st, stop=sp_)
                        nc.tensor.matmul(p2, wv01_slice(sl, fo, 1, ho),
                                         xg_slice(ho, t0, tl),
                                         start=st, stop=sp_)
                for (fo, t0, tl, p1, p2) in grp:
                    glu_tail(fo, t0, tl, p1, p2)

            merged_pass(0, ch0, 0)
            merged_pass(1, ch1, S)

            # --- Phase A steady state ---
            for fo in range(2, FO):
                for sl, chs in ((0, ch0), (1, ch1)):
                    wv = load_wv(sl, fo)
                    for (t0, tl) in chs:
                        p1 = pp.tile([P, 512], F32, tag="pp",
                                     name="p1")[:, :tl]
                        p2 = pp.tile([P, 512], F32, tag="pp",
                                     name="p2")[:, :tl]
                        for ho in range(HO):
                            st, sp_ = (ho == 0), (ho == HO - 1)
                            nc.tensor.matmul(p1, wv[:, 0, ho, :],
                                             xg_slice(ho, t0, tl),
                                             start=st, stop=sp_)
                            nc.tensor.matmul(p2, wv[:, 1, ho, :],
                                             xg_slice(ho, t0, tl),
                                             start=st, stop=sp_)
                        glu_tail(fo, t0, tl, p1, p2)
                if fo == 3:
                    # w2/cb loads pinned behind fo3's h so their
                    # transfers stay clear of the fo2-fo3 wv stream
                    issue_phaseb_loads()

            # gates arrive as logit diffs; sigmoid on the Act engine,
            # pinned after the last silu via a WAW copy on cbs -- the
            # act-table swap (2x 1283ns) then hides under phase B's
            # matmuls instead of blocking the mid-phase silu queue.
            nc.scalar.copy(cbs[:, 0:1], h[:, FO - 1, 0:1])
            nc.scalar.activation(cbs[:], cbt[:],
                                 mybir.ActivationFunctionType.Sigmoid)

            # --- Phase B: yT[hb] = (sum_fo w2[fo,hb]^T h[fo]) * cb ---
            allch = [(0, t0, tl) for (t0, tl) in ch0] + \
                    [(1, t0, tl) for (t0, tl) in ch1]
            # Split the final chunk so only a ~16-token mul+store trails
            # the very last matmul; the 2nd-to-last store rides SWDGE
            # (separate generator) so the final HWDGE gen never queues.
            (sl_l, t0_l, tl_l) = allch[-1]
            if tl_l > 24:
                allch = allch[:-1] + [(sl_l, t0_l, tl_l - 16),
                                      (sl_l, t0_l + tl_l - 16, 16)]
            for hb in range(HO):
                yt = yp.tile([P, CAL], BF16, tag="yt", name=f"yt{hb}")
                last_hb = (hb == HO - 1)
                for ci, (sl, t0, tl) in enumerate(allch):
                    psy = pp.tile([P, 512], F32, tag="pp",
                                  name="psy")[:, :tl]
                    for fo in range(FO):
                        nc.tensor.matmul(psy, w2s[sl][fo][:, hb, :],
                                         h[:, fo, t0:t0 + tl],
                                         start=(fo == 0),
                                         stop=(fo == FO - 1))
                    nc.vector.tensor_mul(yt[:, t0:t0 + tl], psy,
                                         cbs[:, t0:t0 + tl])
                    if last_hb:
                        q = (nc.gpsimd if ci == len(allch) - 2
                             else nc.sync)
                        q.dma_start(
                            y.ap()[hb * P:(hb + 1) * P, t0:t0 + tl],
                            yt[:, t0:t0 + tl])
                if not last_hb:
                    nc.sync.dma_start(y.ap()[hb * P:(hb + 1) * P, 0:CAL],
                                      yt[:])
    nc.compile()
    return nc


def _solve_slots(ns):
    """Find min-CAL two-slot packing: 8 slots of size s, 8 of size r,
    partition into 8 groups with capacity >= n_e.  Returns
    (CAL, s, assignment) with assignment[e] = list of slot sizes, or
    None if nothing beats the single-slot scheme."""
    from functools import lru_cache
    order = sorted(range(E), key=lambda e: -ns[e])
    nss = [ns[e] for e in order]
    single = ((max(ns) + 7) // 8) * 8
    for CAL in range(((sum(ns) + E - 1) // E + 3) // 4 * 4, single, 4):
        for s in range((CAL // 2 + 3) // 4 * 4, CAL - 256, 4):
            r = CAL - s

            @lru_cache(maxsize=None)
            def go(i, sl, rl):
                if i == E:
                    return () if (sl, rl) == (0, 0) else None
                for ks in range(min(2, sl) + 1):
                    for kr in range(min(3, rl) + 1):
                        if ks + kr == 0:
                            continue
                        if ks * s + kr * r >= nss[i]:
                            rest = go(i + 1, sl - ks, rl - kr)
                            if rest is not None:
                                return ((ks, kr),) + rest
                return None

            sol = go(0, 8, 8)
            if sol is not None:
                assign = [None] * E
                for i, (ks, kr) in enumerate(sol):
                    assign[order[i]] = [s] * ks + [r] * kr
                return CAL, s, assign
    return None


def _get_nc(key, builder):
    if key not in _NC_CACHE:
        _NC_CACHE[key] = builder()
    return _NC_CACHE[key]


def _tile_weights(w1, v1, w2):
    """Pre-tile expert weights (bf16) for large-descriptor DMA.

    wvt:  [E, FO, 128(h), 2, HO, 128(f)]  (w1/v1 lhsT tiles, interleaved)
    w2bt: [E, FO, 128(f), HO, 128(h)]     (lhsT tiles of the [F, H] mats)
    """
    key = (w1.shape, w1.dtype.str, w1[0, 0, :4].tobytes(), w2[0, 0, :4].tobytes(),
           v1[0, 0, :4].tobytes(), float(w1[-1, -1, -1]), float(w2[-1, -1, -1]))
    if key in _W_CACHE:
        return _W_CACHE[key]
    # w1[e] is [F, H]; lhsT tile (fo): [p_h, ho, q_f] = w1[e][fo*128+q, ho*128+p]
    w1t = w1.reshape(E, FO, P, HO, P).transpose(0, 1, 4, 3, 2)
    v1t = v1.reshape(E, FO, P, HO, P).transpose(0, 1, 4, 3, 2)
    wvt = np.ascontiguousarray(
        np.stack([w1t, v1t], axis=3).astype(BF_NP))  # [E,FO,P,2,HO,P]
    # w2[e] is [F, H]; lhsT tile (fo, hb): [p_f, j_h] = w2[e][fo*128+p, hb*128+j]
    w2bt = np.ascontiguousarray(w2.reshape(E, FO, P, HO, P).astype(BF_NP))
    # static part of the packed head DMA: [w1(fo0,ho0) | v1(fo0,ho0)]
    hdw = np.ascontiguousarray(
        wvt[:, 0, :, :, 0, :].reshape(E, P, 2 * P))  # [E, P, 256]
    # fo{0,1} x {w1,v1} ho0 / ho1 packs for the hd/hd2 head DMAs:
    # [E, P, 4*128] laid out (fo, m, q)
    hdw01 = [np.ascontiguousarray(
        wvt[:, 0:2, :, :, ho, :].transpose(0, 2, 1, 3, 4).reshape(
            E, P, 4 * P)) for ho in (0, 1)]
    # fo{0,1} 2-ho bundles for the head stream, host-packed flat so each
    # is a plain 2-D DMA: [E, HO//2, P, (fo, m, h, q) = 1024]
    wvp = np.ascontiguousarray(
        wvt[:, 0:2].reshape(E, 2, P, 2, HO // 2, 2, P)
        .transpose(0, 4, 2, 1, 3, 5, 6).reshape(E, HO // 2, P, 8 * P))
    _W_CACHE.clear()
    _W_CACHE[key] = (wvt, w2bt, hdw, hdw01, wvp)
    return wvt, w2bt, hdw, hdw01, wvp


def kernel(x, router_w, w1, v1, w2):
    x = np.asarray(x, dtype=np.float32)
    router_w = np.asarray(router_w, dtype=np.float32)
    w1 = np.asarray(w1, dtype=np.float32)
    v1 = np.asarray(v1, dtype=np.float32)
    w2 = np.asarray(w2, dtype=np.float32)

    xf = x.reshape(T, H)
    xT = np.ascontiguousarray(xf.T)  # [H, T] fp32 (router)
    xT16 = xT.astype(BF_NP)          # [H, T] bf16 (expert gather)
    rwT = np.ascontiguousarray(router_w.T)  # [H, E]

    # ---- Phase 1: router on device (data-parallel over tokens) ----
    nc1 = _get_nc("router", _build_router)
    in1 = [{"xT": np.ascontiguousarray(xT[:, i * TSH:(i + 1) * TSH]),
            "rwT": rwT}
           for i in range(NCORES)]
    r1 = run_bass_kernel_spmd(nc1, in1, core_ids=list(range(NCORES)))
    c = np.concatenate([r["c"] for r in r1.results], axis=0)  # [T, E]

    # ---- Host dispatch: gather tokens per expert (data movement only) ----
    idxs = [np.flatnonzero(c[:, e] != 0.0) for e in range(E)]
    maxc = max(len(ix) for ix in idxs)
    # Per-launch capacity; >1280 tokens per expert (never happens with
    # balanced routing) is handled by running the same NEFF multiple times.
    C = max(1152, min(1280, ((maxc + 127) // 128) * 128))
    nseg = (maxc + C - 1) // C

    wvt, w2bt, hdw, hdw01, wvp = _tile_weights(w1, v1, w2)

    out = np.zeros((T, H), np.float32)

    ns = [len(ix) for ix in idxs]
    sol = _solve_slots(ns) if nseg == 1 else None
    if sol is not None:
        # the two-slot prologue holds every chunk's psum pair at once:
        # at most 4 chunks total fit the 8 psum banks
        CAL, S, assign = sol
        if (len(_slot_chunks(0, S)) +
                len(_slot_chunks(S, CAL - S)) > 4):
            sol = None
    if sol is not None:
        CAL, S, assign = sol
        C2 = max(C, ((CAL + 127) // 128) * 128)
        ch0 = _slot_chunks(0, S)
        c0w = ch0[0][1]
        # map expert slot-size lists onto physical (core, slot) pairs
        s_pool = [(k, 0) for k in range(NCORES)]   # slot0 has size S
        r_pool = [(k, 1) for k in range(NCORES)]   # slot1 has size CAL-S
        core_slots = [[None, None] for _ in range(NCORES)]  # (e, tokens)
        for e in range(E):
            toks = idxs[e]
            off = 0
            for sz in assign[e]:
                pool = s_pool if (sz == S and s_pool) else r_pool
                k, sl = pool.pop()
                take = toks[off:off + sz]
                off += len(take)
                core_slots[k][sl] = (e, take)
            assert off >= len(toks), (e, off, len(toks))
        for k in range(NCORES):
            for sl in range(2):
                if core_slots[k][sl] is None:
                    core_slots[k][sl] = (0, np.zeros((0,), np.int64))
        nc2 = _get_nc(("expert2", C2, CAL, S),
                      lambda: _build_expert2(C2, CAL, S))
        in2 = []
        for k in range(NCORES):
            (ea, ta), (eb, tb) = core_slots[k]
            xgT = np.zeros((H, C2), BF_NP)
            xgT[:, :len(ta)] = xT16[:, ta]
            xgT[:, S:S + len(tb)] = xT16[:, tb]
            cge = np.zeros((CAL,), np.float32)
            cge[:len(ta)] = c[ta, ea]
            cge[S:S + len(tb)] = c[tb, eb]
            cb = np.ascontiguousarray(np.broadcast_to(cge, (P, CAL)))
            # hd  = [wv fo{0,1} ho0 | xg row-block 0, slot0 cols]
            # hd2 = [wv fo{0,1} ho1 | xg row-blocks 1,2, slot0 cols]
            hd = np.concatenate([hdw01[0][ea], xgT[0:P, 0:S]], axis=1)
            hd2 = np.concatenate([hdw01[1][ea], xgT[P:2 * P, 0:S],
                                  xgT[2 * P:3 * P, 0:S]], axis=1)
            in2.append({"hd": np.ascontiguousarray(hd),
                        "hd2": np.ascontiguousarray(hd2), "xgT": xgT,
                        "cb": cb,
                        "wvh0": np.ascontiguousarray(wvp[ea][1:]),
                        "wvh1": np.ascontiguousarray(wvp[eb]),
                        "wvt": np.ascontiguousarray(
                            np.stack([wvt[ea], wvt[eb]])),
                        "w2t": np.ascontiguousarray(
                            np.stack([w2bt[ea], w2bt[eb]]))})
        r2 = run_bass_kernel_spmd(nc2, in2, core_ids=list(range(NCORES)))
        for k in range(NCORES):
            (ea, ta), (eb, tb) = core_slots[k]
            yT = r2.results[k]["y"]  # [H, C2] bf16
            if len(ta):
                out[ta] += yT[:, :len(ta)].T.astype(np.float32)
            if len(tb):
                out[tb] += yT[:, S:S + len(tb)].T.astype(np.float32)
        return out.reshape(x.shape)

    for seg in range(nseg):
        segixs = [idxs[e][seg * C:(seg + 1) * C] for e in range(E)]
        CA = max(1, max(len(ix) for ix in segixs))  # exact active count
        CAL = min(C, ((CA + 7) // 8) * 8)
        c0w = min(512, CAL)
        nc2 = _get_nc(("expert", C, CAL), lambda: _build_expert(C, CAL))
        in2 = []
        for e in range(E):
            ix = segixs[e]
            xgT = np.zeros((H, C), BF_NP)
            xgT[:, :len(ix)] = xT16[:, ix]
            cge = np.zeros((CAL,), np.float32)
            cge[:len(ix)] = c[ix, e]
            cb = np.ascontiguousarray(np.broadcast_to(cge, (P, CAL)))
            hd = np.concatenate([hdw[e], xgT[0:P, 0:c0w]], axis=1)
            in2.append({"hd": np.ascontiguousarray(hd), "xgT": xgT,
                        "cb": cb, "wvt": wvt[e], "w2t": w2bt[e]})
        r2 = run_bass_kernel_spmd(nc2, in2, core_ids=list(range(NCORES)))
        # ---- Host combine: scatter-add per-expert outputs ----
        for e in range(E):
            ix = segixs[e]
            yT = r2.results[e]["y"]  # [H, C] bf16
            out[ix] += yT[:, :len(ix)].T.astype(np.float32)
    return out.reshape(x.shape)

